# revision 12
# baseline (speedup 1.0000x reference)
"""GNN message-passing (NeuralSheaf) Trainium2 kernel.

Model (per batch b):
    h    = relu(x @ nw1 + nb1) @ nw2 + nb2                       # node MLP
    e    = relu(concat(x[u], x[v]) @ ew1 + eb1) @ ew2 + eb2      # edge MLP
    msg  = zeros.at[u].add(e).at[v].add(e)                       # scatter-add
    out  = (h + msg) @ dw + db

Strategy (8 cores, edge-parallel):
  - Edges sharded equally across cores (no sorting needed).
  - dw is folded in before the scatter: scatter(e) @ dw == scatter(e @ dw),
    so we scatter 64-dim messages (both batches packed -> 128 cols).
  - Per 512-edge tile: indirect-DMA row-gather of x_cat[u], x_cat[v]
    (x_cat[n] = [x[0,n,:] | x[1,n,:]], bf16, 256B rows), PE-transpose to
    feature-major, 2-layer MLP via matmuls (split-K over u/v halves),
    then e@dw giving row-major 64-dim messages per batch.
  - Messages are indirect-scattered to a destination-sorted, range-padded
    DRAM stream: each edge's message is written twice (its u-slot and its
    v-slot).  Slot positions are computed on host (index/descriptor prep).
  - Scatter phase: for each 128-node range, read its contribution blocks,
    build a one-hot selector S on DVE (is_equal vs iota), and
    matmul-accumulate S^T @ C into PSUM [128 nodes, 128 (b0|b1 feats)].
  - ReduceScatter(add, bf16) across the 8 cores -> per-core node shard.
  - Node MLP (overlaps the collective) + final add + db -> fp32 output.

kernel() takes FULL inputs and returns the FULL [B, N, IN] fp32 output.
"""

import math
import numpy as np
import ml_dtypes

P = 128
CORES = 8
RT = 512            # edge rows per MLP tile
NSUB = RT // P      # 4 sub-blocks per tile

_BF16 = ml_dtypes.bfloat16


# ----------------------------------------------------------------------------
# Host-side preparation (sharding + index/descriptor prep)
# ----------------------------------------------------------------------------

def _transposed_fill(seq, T):
    """[T*RT] -> [P, T*NSUB] with tile[p, t*NSUB+s] = seq[t*RT + s*P + p]."""
    return np.ascontiguousarray(
        seq.reshape(T, NSUB, P).transpose(2, 0, 1).reshape(P, T * NSUB)
    )


def _prep(x, edges, ew1, eb1, ew2, eb2, nw1, nb1, nw2, nb2, dw, db):
    B, N, IN = x.shape
    E = edges.shape[0]
    HID = ew1.shape[1]
    OUT = ew2.shape[1]
    assert B == 2 and 2 * IN == ew1.shape[0]

    Ec = E // CORES
    T = math.ceil(Ec / RT)
    Ec_pad = T * RT
    NR = math.ceil(N / P)                      # node ranges
    RSROWS = math.ceil(NR * P / CORES) * CORES
    NS = RSROWS // CORES                       # per-core node-shard rows

    x64 = np.asarray(x, np.float32)
    x_cat = np.concatenate([x64[0], x64[1]], axis=1)       # [N, 2*IN]
    x_cat_bf = x_cat.astype(_BF16)

    edges = np.asarray(edges)
    u_all = edges[:, 0].astype(np.int64)
    v_all = edges[:, 1].astype(np.int64)

    # --- contribution counts per (core, range) to fix uniform block counts ---
    per_core = []
    c_mat = np.zeros((CORES, NR), np.int64)
    for k in range(CORES):
        u = u_all[k * Ec:(k + 1) * Ec]
        v = v_all[k * Ec:(k + 1) * Ec]
        dests = np.concatenate([u, v])
        r = dests // P
        c_mat[k] = np.bincount(r, minlength=NR)
        per_core.append((u, v, dests, r))
    b_r = np.maximum(np.ceil(c_mat / P).astype(np.int64).max(axis=0), 1)  # [NR]
    blk_start = np.concatenate([[0], np.cumsum(b_r)])      # [NR+1]
    NBLK = int(blk_start[-1])
    NSLOT = NBLK * P
    # tail region for dummy (padding) scatter targets
    TAIL = 2 * (Ec_pad - Ec) + P
    NSLOT_pad = NSLOT + ((TAIL + 2 * P + P - 1) // P) * P

    # padding (unassigned) slot counts -> uniform PADCOLS
    missing_counts = []
    for k in range(CORES):
        m = int(NSLOT - 2 * Ec)
        missing_counts.append(m)
    PADCOLS = max(math.ceil(m / P) for m in missing_counts)

    in_maps = []
    for k in range(CORES):
        u, v, dests, r = per_core[k]
        M = dests.shape[0]                                  # 2*Ec
        order = np.argsort(r, kind="stable")
        r_sorted = r[order]
        c_r = c_mat[k]
        range_first = np.concatenate([[0], np.cumsum(c_r)])[:-1]
        within = np.arange(M, dtype=np.int64) - np.repeat(range_first, c_r)
        slot_sorted = blk_start[r_sorted] * P + within
        slot = np.empty(M, np.int64)
        slot[order] = slot_sorted
        pos_u = slot[:Ec]
        pos_v = slot[Ec:]

        # sdest [P, NBLK]: local node offset for each assigned slot, -1 pads
        sdest = np.full((P, NBLK), -1.0, np.float32)
        sp = slot % P
        sb = slot // P
        sdest[sp, sb] = (dests - r * P).astype(np.float32)

        # unassigned slots in [0, NSLOT) -> zero-fill targets
        assigned = np.zeros(NSLOT, bool)
        assigned[slot] = True
        missing = np.nonzero(~assigned)[0]
        need = PADCOLS * P
        if missing.shape[0] < need:
            fill = NSLOT + np.arange(need - missing.shape[0]) % P
            missing = np.concatenate([missing, fill])
        padidx = np.ascontiguousarray(
            missing[:need].reshape(PADCOLS, P).T.astype(np.int32))

        # pad edges -> gather node 0, scatter to distinct tail slots
        npad = Ec_pad - Ec
        u_p = np.concatenate([u, np.zeros(npad, np.int64)])
        v_p = np.concatenate([v, np.zeros(npad, np.int64)])
        tail0 = NSLOT + P
        pos_u_p = np.concatenate([pos_u, tail0 + np.arange(npad)])
        pos_v_p = np.concatenate([pos_v, tail0 + npad + np.arange(npad)])

        uidx = _transposed_fill(u_p.astype(np.int32), T)
        vidx = _transposed_fill(v_p.astype(np.int32), T)
        upos = _transposed_fill(pos_u_p.astype(np.int32), T)
        vpos = _transposed_fill(pos_v_p.astype(np.int32), T)

        # node shard (transposed), zero-padded at the global tail
        w0 = k * NS
        xw = np.zeros((NS, 2 * IN), np.float32)
        w1 = min(w0 + NS, N)
        if w1 > w0:
            xw[: w1 - w0] = x_cat[w0:w1]
        xT_shard = np.ascontiguousarray(xw.T).astype(_BF16)   # [2*IN, NS]

        bias_pack = np.zeros((P, 6), np.float32)
        bias_pack[:, 0] = eb1[:P]
        bias_pack[:, 1] = eb1[P:]
        bias_pack[:, 2] = eb2
        bias_pack[:, 3] = nb1[:P]
        bias_pack[:, 4] = nb1[P:]
        bias_pack[:, 5] = nb2
        db_row = np.broadcast_to(
            np.tile(np.asarray(db, np.float32), 2)[None, :],
            (P, 2 * IN)).copy()                                # [P, 2*IN]

        iota = np.broadcast_to(
            np.arange(P, dtype=np.float32)[None, :], (P, P)).copy()

        in_maps.append({
            "x_cat": x_cat_bf,
            "xT_shard": xT_shard,
            "ew1": np.asarray(ew1, np.float32).astype(_BF16),
            "ew2": np.asarray(ew2, np.float32).astype(_BF16),
            "nw1": np.asarray(nw1, np.float32).astype(_BF16),
            "nw2": np.asarray(nw2, np.float32).astype(_BF16),
            "dww": np.asarray(dw, np.float32).astype(_BF16),
            "bias_pack": bias_pack,
            "db_row": np.ascontiguousarray(db_row, np.float32),
            "iota": np.ascontiguousarray(iota, np.float32),
            "uidx": uidx, "vidx": vidx, "upos": upos, "vpos": vpos,
            "sdest": sdest, "padidx": padidx,
        })

    meta = dict(B=B, N=N, IN=IN, HID=HID, OUT=OUT, E=E, Ec=Ec, T=T,
                NR=NR, NS=NS, RSROWS=RSROWS, NBLK=NBLK,
                NSLOT_pad=NSLOT_pad, PADCOLS=PADCOLS,
                b_r=[int(x) for x in b_r], blk_start=[int(x) for x in blk_start])
    return in_maps, meta


# ----------------------------------------------------------------------------
# Bass kernel
# ----------------------------------------------------------------------------

def _build(meta):
    import concourse.bacc as bacc
    import concourse.bass as bass
    import concourse.tile as tile
    import concourse.mybir as mybir
    from concourse.masks import make_identity

    dt = mybir.dt
    AF = mybir.ActivationFunctionType
    IOX = bass.IndirectOffsetOnAxis

    B = meta["B"]; IN = meta["IN"]; HID = meta["HID"]; OUT = meta["OUT"]
    T = meta["T"]; NR = meta["NR"]; NS = meta["NS"]; RSROWS = meta["RSROWS"]
    NBLK = meta["NBLK"]; NSLOT_pad = meta["NSLOT_pad"]; PADCOLS = meta["PADCOLS"]
    b_r = meta["b_r"]; blk_start = meta["blk_start"]
    IN2 = 2 * IN
    MH = HID // P       # hid M-halves (2)
    assert HID == 2 * P and IN2 == P and OUT == P and IN == 64

    nc = bacc.Bacc("TRN2", target_bir_lowering=False, debug=False,
                   enable_asserts=False, num_devices=CORES)

    def din(name, shape, dty):
        return nc.dram_tensor(name, list(shape), dty, kind="ExternalInput").ap()

    x_cat = din("x_cat", (meta["N"], IN2), dt.bfloat16)
    xT_shard = din("xT_shard", (IN2, NS), dt.bfloat16)
    ew1 = din("ew1", (IN2, HID), dt.bfloat16)
    ew2 = din("ew2", (HID, OUT), dt.bfloat16)
    nw1 = din("nw1", (IN, HID), dt.bfloat16)
    nw2 = din("nw2", (HID, OUT), dt.bfloat16)
    dww = din("dww", (OUT, IN), dt.bfloat16)
    bias_pack = din("bias_pack", (P, 6), dt.float32)
    db_row = din("db_row", (P, IN2), dt.float32)
    iota_in = din("iota", (P, P), dt.float32)
    uidx = din("uidx", (P, T * NSUB), dt.int32)
    vidx = din("vidx", (P, T * NSUB), dt.int32)
    upos = din("upos", (P, T * NSUB), dt.int32)
    vpos = din("vpos", (P, T * NSUB), dt.int32)
    sdest = din("sdest", (P, NBLK), dt.float32)
    padidx = din("padidx", (P, PADCOLS), dt.int32)

    out = nc.dram_tensor("out", [NS, IN2], dt.float32, kind="ExternalOutput").ap()
    debug = meta.get("debug", False)
    if debug:
        dbg_emsg = nc.dram_tensor("dbg_emsg", [NSLOT_pad, IN2], dt.bfloat16,
                                  kind="ExternalOutput").ap()
        dbg_rsin = nc.dram_tensor("dbg_rsin", [RSROWS, IN2], dt.bfloat16,
                                  kind="ExternalOutput").ap()

    with tile.TileContext(nc) as tc:
        with (
            tc.tile_pool(name="const", bufs=1) as cpool,
            tc.tile_pool(name="dram", bufs=1, space="DRAM") as dpool,
        ):
            emsg = dpool.tile([NSLOT_pad, IN2], dt.bfloat16)
            rs_in = dpool.tile([RSROWS, IN2], dt.bfloat16)
            rs_out = dpool.tile([NS, IN2], dt.bfloat16)

            # ---- constants to SBUF ----
            # ew1_a: as-is (u rows 0:64, v rows 64:128); ew1_b: swapped halves.
            # Needed because matmul requires lhsT/rhs at the same base
            # partition: batch0 reads Tu/Tv partitions 0:64, batch1 64:128.
            ew1_a = cpool.tile([IN2, HID], dt.bfloat16)
            nc.sync.dma_start(ew1_a[:], ew1[:])
            ew1_b = cpool.tile([IN2, HID], dt.bfloat16)
            nc.sync.dma_start(ew1_b[0:IN, :], ew1[IN:IN2, :])
            nc.sync.dma_start(ew1_b[IN:IN2, :], ew1[0:IN, :])
            ew2_sb = [cpool.tile([P, OUT], dt.bfloat16, tag=f"ew2_{k}", name=f"ew2_sb{k}")
                      for k in range(MH)]
            for k in range(MH):
                nc.sync.dma_start(ew2_sb[k][:], ew2[k * P:(k + 1) * P, :])
            # nw1 duplicated on both partition halves (batch0 / batch1 rhs)
            nw1_sb = cpool.tile([IN2, HID], dt.bfloat16)
            nc.sync.dma_start(nw1_sb[0:IN, :], nw1[:])
            nc.sync.dma_start(nw1_sb[IN:IN2, :], nw1[:])
            nw2_sb = [cpool.tile([P, OUT], dt.bfloat16, tag=f"nw2_{k}", name=f"nw2_sb{k}")
                      for k in range(MH)]
            for k in range(MH):
                nc.sync.dma_start(nw2_sb[k][:], nw2[k * P:(k + 1) * P, :])
            dw_sb = cpool.tile([OUT, IN], dt.bfloat16)
            nc.sync.dma_start(dw_sb[:], dww[:])
            bias_sb = cpool.tile([P, 6], dt.float32)
            nc.sync.dma_start(bias_sb[:], bias_pack[:])
            db_sb = cpool.tile([P, IN2], dt.float32)
            nc.sync.dma_start(db_sb[:], db_row[:])
            iota_sb = cpool.tile([P, P], dt.float32)
            nc.sync.dma_start(iota_sb[:], iota_in[:])
            uidx_sb = cpool.tile([P, T * NSUB], dt.int32)
            nc.sync.dma_start(uidx_sb[:], uidx[:])
            vidx_sb = cpool.tile([P, T * NSUB], dt.int32)
            nc.sync.dma_start(vidx_sb[:], vidx[:])
            upos_sb = cpool.tile([P, T * NSUB], dt.int32)
            nc.sync.dma_start(upos_sb[:], upos[:])
            vpos_sb = cpool.tile([P, T * NSUB], dt.int32)
            nc.sync.dma_start(vpos_sb[:], vpos[:])
            sdest_sb = cpool.tile([P, NBLK], dt.float32)
            nc.sync.dma_start(sdest_sb[:], sdest[:])
            padidx_sb = cpool.tile([P, PADCOLS], dt.int32)
            nc.sync.dma_start(padidx_sb[:], padidx[:])
            ident_bf = cpool.tile([P, P], dt.bfloat16)
            make_identity(nc, ident_bf[:])
            zero_sb = cpool.tile([P, NSUB, IN2], dt.bfloat16)
            nc.gpsimd.memset(zero_sb[:], 0)

            # ---- phase 0: zero the unassigned scatter slots ----
            # (HW indirect DMA supports one index per partition per call)
            for c in range(PADCOLS):
                nc.gpsimd.indirect_dma_start(
                    out=emsg[:],
                    out_offset=IOX(ap=padidx_sb[:, c:c + 1], axis=0),
                    in_=zero_sb[:, 0, :], in_offset=None)

            # ---- phase 1: edge MLP + message position-scatter ----
            with (
                tc.tile_pool(name="p1", bufs=3) as p1,
                tc.tile_pool(name="p1h", bufs=2) as p1h,
                tc.tile_pool(name="ps_tr", bufs=2, space="PSUM") as ps_tr,
                tc.tile_pool(name="ps_h", bufs=2, space="PSUM") as ps_h,
                tc.tile_pool(name="ps_o", bufs=2, space="PSUM") as ps_o,
                tc.tile_pool(name="ps_dw", bufs=2, space="PSUM") as ps_dw,
            ):
                for t in range(T):
                    g = [p1.tile([P, NSUB, IN2], dt.bfloat16, tag="g_u", name="g_u"),
                         p1.tile([P, NSUB, IN2], dt.bfloat16, tag="g_v", name="g_v")]
                    for s in range(NSUB):
                        col = t * NSUB + s
                        nc.gpsimd.indirect_dma_start(
                            out=g[0][:, s, :], out_offset=None, in_=x_cat[:],
                            in_offset=IOX(ap=uidx_sb[:, col:col + 1], axis=0))
                        nc.gpsimd.indirect_dma_start(
                            out=g[1][:, s, :], out_offset=None, in_=x_cat[:],
                            in_offset=IOX(ap=vidx_sb[:, col:col + 1], axis=0))
                    # transpose to feature-major Tu/Tv [IN2, RT]
                    tuv = [p1.tile([P, RT], dt.bfloat16, tag="t_u", name="t_u"),
                           p1.tile([P, RT], dt.bfloat16, tag="t_v", name="t_v")]
                    for e in range(2):
                        pt = ps_tr.tile([P, NSUB, P], dt.bfloat16, tag="tr",
                                        name="pt")
                        for s in range(NSUB):
                            nc.tensor.transpose(pt[:, s, :], g[e][:, s, :],
                                                ident_bf[:])
                        for s in range(NSUB):
                            if s % 2 == 0:
                                nc.vector.tensor_copy(
                                    tuv[e][:, s * P:(s + 1) * P], pt[:, s, :])
                            else:
                                nc.scalar.activation(
                                    tuv[e][:, s * P:(s + 1) * P], pt[:, s, :],
                                    AF.Copy)
                    # stage 1: H1^T[b][m] = relu(ew1^T @ [xu;xv]) + eb1
                    h1 = {}
                    for b in range(B):
                        for m in range(MH):
                            ph = ps_h.tile([P, RT], dt.float32, tag="h")
                            lo, hi = b * IN, (b + 1) * IN
                            # u-part weights at this batch's partition base
                            w_u = ew1_a if b == 0 else ew1_b
                            w_v = ew1_b if b == 0 else ew1_a
                            nc.tensor.matmul(
                                ph[:], w_u[lo:hi, m * P:(m + 1) * P],
                                tuv[0][lo:hi, :], start=True, stop=False)
                            nc.tensor.matmul(
                                ph[:], w_v[lo:hi, m * P:(m + 1) * P],
                                tuv[1][lo:hi, :], start=False, stop=True)
                            hs = p1h.tile([P, RT], dt.bfloat16,
                                          tag=f"h1_{b}_{m}")
                            nc.scalar.activation(hs[:], ph[:], AF.Relu,
                                                 bias=bias_sb[:, m:m + 1])
                            h1[(b, m)] = hs
                    # stage 2 + dw fold + staging
                    staging = p1.tile([P, NSUB, IN2], dt.bfloat16, tag="stg")
                    for b in range(B):
                        po = ps_o.tile([P, RT], dt.float32, tag="o")
                        for k in range(MH):
                            nc.tensor.matmul(po[:], ew2_sb[k][:],
                                             h1[(b, k)][:],
                                             start=(k == 0), stop=(k == MH - 1))
                        ob = p1h.tile([P, RT], dt.bfloat16, tag=f"o_{b}")
                        nc.scalar.activation(ob[:], po[:], AF.Identity,
                                             bias=bias_sb[:, 2:3])
                        for s in range(NSUB):
                            pd = ps_dw.tile([P, IN], dt.float32, tag="dwp")
                            nc.tensor.matmul(pd[:],
                                             ob[:, s * P:(s + 1) * P],
                                             dw_sb[:], start=True, stop=True)
                            nc.vector.tensor_copy(
                                staging[:, s, b * IN:(b + 1) * IN], pd[:])
                    # scatter messages to u-slots and v-slots
                    for s in range(NSUB):
                        col = t * NSUB + s
                        nc.gpsimd.indirect_dma_start(
                            out=emsg[:],
                            out_offset=IOX(ap=upos_sb[:, col:col + 1], axis=0),
                            in_=staging[:, s, :], in_offset=None)
                        nc.gpsimd.indirect_dma_start(
                            out=emsg[:],
                            out_offset=IOX(ap=vpos_sb[:, col:col + 1], axis=0),
                            in_=staging[:, s, :], in_offset=None)

            # ---- phase 2: per-range segment sum via one-hot matmul ----
            with (
                tc.tile_pool(name="p2", bufs=4) as p2,
                tc.tile_pool(name="p2s", bufs=4) as p2s,
                tc.tile_pool(name="ps_acc", bufs=2, space="PSUM") as ps_acc,
            ):
                for r in range(NR):
                    br = b_r[r]
                    pacc = ps_acc.tile([P, IN2], dt.float32, tag="acc")
                    for b in range(br):
                        blk = blk_start[r] + b
                        c = p2.tile([P, IN2], dt.bfloat16, tag="c")
                        nc.sync.dma_start(
                            c[:], emsg[blk * P:(blk + 1) * P, :])
                        s_t = p2s.tile([P, P], dt.bfloat16, tag="s")
                        nc.vector.tensor_tensor(
                            out=s_t[:],
                            in0=sdest_sb[:, blk:blk + 1].to_broadcast([P, P]),
                            in1=iota_sb[:],
                            op=mybir.AluOpType.is_equal)
                        nc.tensor.matmul(pacc[:], s_t[:], c[:],
                                         start=(b == 0), stop=(b == br - 1))
                    rsb = p2s.tile([P, IN2], dt.bfloat16, tag="rsb")
                    nc.scalar.activation(rsb[:], pacc[:], AF.Copy)
                    nc.sync.dma_start(rs_in[r * P:(r + 1) * P, :], rsb[:])

            if debug:
                nc.sync.dma_start(dbg_emsg[:], emsg[:])
                nc.sync.dma_start(dbg_rsin[:], rs_in[:])

            # ---- phase 3: reduce-scatter across cores ----
            nc.gpsimd.collective_compute(
                "ReduceScatter", mybir.AluOpType.add,
                replica_groups=[list(range(CORES))],
                ins=[rs_in.opt()], outs=[rs_out.opt()])

            # ---- phase 4: node MLP + final add ----
            NT = math.ceil(NS / RT)
            with (
                tc.tile_pool(name="p4", bufs=2) as p4,
                tc.tile_pool(name="ps_n", bufs=2, space="PSUM") as ps_n,
                tc.tile_pool(name="ps_nd", bufs=2, space="PSUM") as ps_nd,
            ):
                for t in range(NT):
                    c0 = t * RT
                    R = min(RT, NS - c0)
                    xn = p4.tile([P, RT], dt.bfloat16, tag="xn")
                    nc.sync.dma_start(xn[:, :R], xT_shard[:, c0:c0 + R])
                    h1n = {}
                    for b in range(B):
                        for m in range(MH):
                            ph = ps_n.tile([P, RT], dt.float32, tag="nh")
                            lo, hi = b * IN, (b + 1) * IN
                            nc.tensor.matmul(
                                ph[:, :R], nw1_sb[lo:hi, m * P:(m + 1) * P],
                                xn[lo:hi, :R],
                                start=True, stop=True)
                            hs = p4.tile([P, RT], dt.bfloat16,
                                         tag=f"nh1_{b}_{m}")
                            nc.scalar.activation(hs[:, :R], ph[:, :R], AF.Relu,
                                                 bias=bias_sb[:, 3 + m:4 + m])
                            h1n[(b, m)] = hs
                    hstage = p4.tile([P, NSUB, IN2], dt.float32, tag="hstg")
                    for b in range(B):
                        po = ps_n.tile([P, RT], dt.float32, tag="no")
                        for k in range(MH):
                            nc.tensor.matmul(po[:, :R], nw2_sb[k][:],
                                             h1n[(b, k)][:, :R],
                                             start=(k == 0), stop=(k == MH - 1))
                        ob = p4.tile([P, RT], dt.bfloat16, tag=f"no_{b}")
                        nc.scalar.activation(ob[:, :R], po[:, :R], AF.Identity,
                                             bias=bias_sb[:, 5:6])
                        for s in range(math.ceil(R / P)):
                            rr = min(P, R - s * P)
                            pd = ps_nd.tile([P, IN], dt.float32, tag="ndw")
                            nc.tensor.matmul(pd[:rr, :],
                                             ob[:, s * P:s * P + rr],
                                             dw_sb[:], start=True, stop=True)
                            nc.vector.tensor_tensor(
                                out=hstage[:rr, s, b * IN:(b + 1) * IN],
                                in0=pd[:rr, :],
                                in1=db_sb[:rr, b * IN:(b + 1) * IN],
                                op=mybir.AluOpType.add)
                    for s in range(math.ceil(R / P)):
                        rr = min(P, R - s * P)
                        gchunk = c0 + s * P
                        rt_ = p4.tile([P, IN2], dt.bfloat16, tag="rt")
                        nc.sync.dma_start(rt_[:rr, :],
                                          rs_out[gchunk:gchunk + rr, :])
                        ot = p4.tile([P, IN2], dt.float32, tag="ot")
                        nc.vector.tensor_tensor(
                            out=ot[:rr, :], in0=hstage[:rr, s, :],
                            in1=rt_[:rr, :], op=mybir.AluOpType.add)
                        nc.sync.dma_start(out[gchunk:gchunk + rr, :],
                                          ot[:rr, :])

    nc.compile()
    return nc


# ----------------------------------------------------------------------------
# Entry point
# ----------------------------------------------------------------------------

_CACHE = {}


def _run(inputs, trace=False):
    in_maps, meta = _prep(**inputs)
    key = (meta["N"], meta["E"], meta["NBLK"], meta["PADCOLS"])
    if key not in _CACHE:
        _CACHE[key] = _build(meta)
    nc = _CACHE[key]
    from concourse.bass_utils import run_bass_kernel_spmd
    res = run_bass_kernel_spmd(nc, in_maps, core_ids=list(range(CORES)),
                               trace=trace)
    B, N, IN = meta["B"], meta["N"], meta["IN"]
    NS = meta["NS"]
    full = np.concatenate([r["out"] for r in res.results], axis=0)[:N]
    outx = np.empty((B, N, IN), np.float32)
    for b in range(B):
        outx[b] = full[:, b * IN:(b + 1) * IN]
    return outx, res


def kernel(**inputs):
    outx, _ = _run(inputs, trace=False)
    return outx
